# revision 25
# baseline (speedup 1.0000x reference)
"""PolyMPNN Trainium2 kernel v3: 4-layer edge-MLP message passing GNN.

Strategy (8 NeuronCores, SPMD single program):
- Nodes sharded contiguously: 6300/core (50400 padded). Each core owns the
  edges whose destination (row) falls in its shard, grouped by 126-node
  windows, split by col parity, padded to 128-edge chunks; chunk schedule
  uniform across cores.
- Per layer: Q = h@W_c computed per group (bf16) -> AllGather; P = h@W_r + b1
  computed per group into a [128, F] table whose rows 126:128 hold the
  edge-feature weights W_e, overlapping the collective.
- Q values fetched per edge with dma_gather over 4 SWDGE queues from the
  bf16 pair table [25200, 128] (256B packets, idx = col>>1, parity picks
  the 64-wide half).
- P + ef term in ONE matmul per chunk: lhsT is a [128, 128e] tile whose rows
  0:126 are the node one-hot (built by a single in-place is_equal over a
  host-replicated rloc block) and rows 126:128 are the edge features.
- Scatter-add by one-hot matmul (agg[65,126] += msg[128e,65].T @ oh_e);
  row 64 (ones col) yields per-node degree for the b2 term.
- Node update: h' = relu(LN(agg@W2 + deg*b2 + skip_b + h@skip_w)), LN in
  feature-on-partition layout using ones-matmul statistics. All matmuls bf16.
"""
import sys

if "/opt/trn_rl_repo" not in sys.path:
    sys.path.insert(0, "/opt/trn_rl_repo")

import numpy as np
import ml_dtypes

BF16 = ml_dtypes.bfloat16

NCORES = 8
N = 50000
NSH = 6300            # nodes per core (= GW * G)
NPAD = NSH * NCORES   # 50400
NPAIR = NPAD // 2     # 25200 node pairs
GW = 126              # node group width (126 + 2 ef rows = 128)
G = NSH // GW         # 50 groups per core
F = 64                # embed
HID = 128             # encoder hidden
L = 4
POLY = 8
TN = 450              # node tile width for matmul passes (14 tiles)
GB = 2                # groups per batch
PH = NSH // 2         # 3150 pairs per core
PH0 = 1600            # pairs in half 0 (node tiles 0..24)
PH1 = PH - PH0        # 1550 pairs in half 1
H0TOT = NCORES * PH0  # 12800


def _wrap_idx(idx_flat: np.ndarray) -> np.ndarray:
    """[n] -> [128, n//16] int16 wrapped (16-lane) + replicated layout."""
    n = len(idx_flat)
    assert n % 16 == 0
    a = idx_flat.reshape(n // 16, 16).T.astype(np.int16)
    return np.ascontiguousarray(np.tile(a, (8, 1)))


def _preprocess(node_features, edge_index, edge_features):
    """Sort/pad edges; build per-core device arrays + shared chunk schedule."""
    rows = edge_index[0].astype(np.int64)
    cols = edge_index[1].astype(np.int64)

    owner = rows // NSH
    lrow = rows % NSH
    grp = lrow // GW
    par = cols & 1

    counts = np.zeros((NCORES, G, 2), np.int64)
    np.add.at(counts, (owner, grp, par), 1)
    Kev = np.ceil(counts[:, :, 0].max(axis=0) / 128).astype(np.int64)
    Kod = np.ceil(counts[:, :, 1].max(axis=0) / 128).astype(np.int64)
    K = Kev + Kod
    C = int(K.sum())

    batches = []
    c0 = 0
    for b0 in range(0, G, GB):
        gs = list(range(b0, min(b0 + GB, G)))
        keb = int(Kev[gs].sum())
        kb = int(K[gs].sum())
        epos, opos = {}, {}
        e_off, o_off = 0, keb
        for g in gs:
            epos[g] = (e_off, e_off + int(Kev[g]))
            opos[g] = (o_off, o_off + int(Kod[g]))
            e_off += int(Kev[g])
            o_off += int(Kod[g])
        cgrp = np.zeros(kb, np.int64)
        for g in gs:
            cgrp[epos[g][0]:epos[g][1]] = g
            cgrp[opos[g][0]:opos[g][1]] = g
        batches.append(dict(groups=gs, c0=c0, kb=kb, keb=keb,
                            epos=epos, opos=opos, cgrp=cgrp))
        c0 += kb
    assert c0 == C

    order = np.lexsort((par, grp, owner))
    srows, scols, sgrp, sowner, spar = (lrow[order], cols[order], grp[order],
                                        owner[order], par[order])
    sef = edge_features[order].astype(np.float32)

    slot_base = np.zeros((NCORES, G, 2), np.int64)
    for b in batches:
        for g in b["groups"]:
            slot_base[:, g, 0] = (b["c0"] + b["epos"][g][0]) * 128
            slot_base[:, g, 1] = (b["c0"] + b["opos"][g][0]) * 128

    key = (sowner * G + sgrp) * 2 + spar
    _, first_idx, key_counts = np.unique(key, return_index=True,
                                         return_counts=True)
    rank = np.arange(len(key), dtype=np.int64)
    rank -= np.repeat(first_idx, key_counts)
    slot = slot_base[sowner, sgrp, spar] + rank

    qidx = np.zeros((NCORES, C * 128), np.int64)
    rloc = np.full((NCORES, 128, C), 999.0, np.float32)
    rflat = np.full((NCORES, C * 128), 999.0, np.float32)
    ef = np.zeros((NCORES, 2, C * 128), np.float32)
    qidx[sowner, slot] = scols >> 1
    lane = slot % 128
    chunk = slot // 128
    rloc[sowner, lane, chunk] = (srows % GW).astype(np.float32)
    rflat[sowner, slot] = (srows % GW).astype(np.float32)
    ef[sowner, 0, slot] = sef[:, 0]
    ef[sowner, 1, slot] = sef[:, 1]


    # rep8: replicated rloc rows (slot-major) in int8; sentinel 127
    r8 = np.where(rflat >= GW, 127, rflat).astype(np.int8)
    rep8 = np.broadcast_to(r8[:, None, :], (NCORES, GW, C * 128)).copy()

    qidx_w = np.zeros((NCORES, 128, C * 8), np.int16)
    for c in range(NCORES):
        for b in batches:
            s, kb = b["c0"], b["kb"]
            qidx_w[c][:, s * 8:(s + kb) * 8] = _wrap_idx(
                qidx[c][s * 128:(s + kb) * 128])

    nf = np.zeros((NPAD, 3), np.float32)
    nf[:N] = node_features
    nf1T = np.zeros((NCORES, 4, NSH), np.float32)
    for c in range(NCORES):
        nf1T[c, 0:3] = nf[c * NSH:(c + 1) * NSH].T
        nf1T[c, 3] = 1.0

    sched = dict(K=K, C=C, batches=batches)
    percore = dict(qidx_w=qidx_w,
                   rloc=rloc.astype(BF16),
                   rep8=rep8,
                   ef=ef.astype(BF16),
                   nf1T=nf1T.astype(BF16))
    return sched, percore


def _build(sched):
    """Build the Bass program for the shared chunk schedule."""
    import concourse.mybir as mybir
    import concourse.tile as tile
    from concourse import bacc

    dt = mybir.dt
    fp = dt.float32
    bf = dt.bfloat16
    AOT = mybir.AluOpType
    ACT = mybir.ActivationFunctionType

    C = sched["C"]
    batches = sched["batches"]
    K = sched["K"]

    nc = bacc.Bacc("TRN2", num_devices=NCORES, num_swdge_queues=4)

    # ---- I/O ----
    nf1T_d = nc.dram_tensor("nf1T", [4, NSH], bf, kind="ExternalInput")
    qidx_d = nc.dram_tensor("qidx", [128, C * 8], dt.int16, kind="ExternalInput")
    rloc_d = nc.dram_tensor("rloc", [128, C], bf, kind="ExternalInput")
    rep8_d = nc.dram_tensor("rep8", [GW, C * 128], dt.int8, kind="ExternalInput")
    ef_d = nc.dram_tensor("ef", [2, C * 128], bf, kind="ExternalInput")
    iotap8_d = nc.dram_tensor("iotap8", [128, 1], dt.int8, kind="ExternalInput")
    iota_d = nc.dram_tensor("iota", [128, GW], bf, kind="ExternalInput")
    iotap_d = nc.dram_tensor("iotap", [128, 1], bf, kind="ExternalInput")
    onesbd_d = nc.dram_tensor("onesbd", [128, 2], bf, kind="ExternalInput")
    ones64_d = nc.dram_tensor("ones64", [1, 64], bf, kind="ExternalInput")
    encw1b_d = nc.dram_tensor("encw1b", [4, HID], bf, kind="ExternalInput")
    encw2_d = nc.dram_tensor("encw2", [HID, F], bf, kind="ExternalInput")
    encb2_d = nc.dram_tensor("encb2", [F, 1], fp, kind="ExternalInput")
    wrb1_d = nc.dram_tensor("wrb1", [L, 65, F], bf, kind="ExternalInput")
    wc_d = nc.dram_tensor("wc", [L, 65, F], bf, kind="ExternalInput")
    web_d = nc.dram_tensor("web", [L, 2, F], bf, kind="ExternalInput")
    webrep_d = nc.dram_tensor("webrep", [L, 2, G * F], bf, kind="ExternalInput")
    w2b_d = nc.dram_tensor("w2b", [L, 65, F], bf, kind="ExternalInput")
    skb_d = nc.dram_tensor("skb", [L, F, 1], fp, kind="ExternalInput")
    skw_d = nc.dram_tensor("skw", [L, F, F], bf, kind="ExternalInput")
    skwb_d = nc.dram_tensor("skwb", [L, 65, F], bf, kind="ExternalInput")
    ident_d = nc.dram_tensor("ident", [128, 128], bf, kind="ExternalInput")
    lng_d = nc.dram_tensor("lng", [L, F, 1], fp, kind="ExternalInput")
    lnb_d = nc.dram_tensor("lnb", [L, F, 1], fp, kind="ExternalInput")
    hw1_d = nc.dram_tensor("hw1", [F, F], bf, kind="ExternalInput")
    hb1_d = nc.dram_tensor("hb1", [F, 1], fp, kind="ExternalInput")
    hw2_d = nc.dram_tensor("hw2", [F, POLY], bf, kind="ExternalInput")
    hb2_d = nc.dram_tensor("hb2", [POLY, 1], fp, kind="ExternalInput")
    outT_d = nc.dram_tensor("outT", [POLY, NSH], fp, kind="ExternalOutput")
    # internal (bf16 pair layout: row j holds nodes 2j, 2j+1)
    q_local = nc.dram_tensor("q_local", [NSH // 2, 2 * F], bf)
    q_full = nc.dram_tensor("q_full", [NPAIR, 2 * F], bf, addr_space="Shared")

    ntiles = [(t * TN, min(TN, NSH - t * TN)) for t in range((NSH + TN - 1) // TN)]
    n128 = [(t * 128, min(128, NSH - t * 128)) for t in range((NSH + 127) // 128)]

    with tile.TileContext(nc) as tc:
        with (
            tc.tile_pool(name="persist", bufs=1) as pp,
            tc.tile_pool(name="wts", bufs=1) as wp,
        ):
            # persistent state
            hT = pp.tile([65, NSH], bf)         # rows 0-63 h, row 64 ones
            aggT = pp.tile([65, NSH], bf)       # rows 0-63 agg, row 64 deg
            PW_all = pp.tile([128, G * F], bf)  # rows 0:126 P_g, 126:128 W_e
            iota_t = pp.tile([128, GW], bf)
            iotap_t = pp.tile([128, 1], bf)
            iotap8_t = pp.tile([128, 1], dt.int8)
            nc.sync.dma_start(out=iotap8_t[:], in_=iotap8_d[:, :])
            ident_t = pp.tile([128, 128], bf)
            nc.sync.dma_start(out=ident_t[:], in_=ident_d[:, :])
            onesbd_t = pp.tile([128, 2], bf)
            ones64_t = pp.tile([1, 64], bf)
            nc.sync.dma_start(out=iota_t[:], in_=iota_d[:, :])
            nc.sync.dma_start(out=iotap_t[:], in_=iotap_d[:, :])
            nc.sync.dma_start(out=onesbd_t[:], in_=onesbd_d[:, :])
            nc.sync.dma_start(out=ones64_t[:], in_=ones64_d[:, :])
            nc.vector.memset(hT[64:65, :], 1.0)
            eps_t = pp.tile([1, 1], fp)
            nc.vector.memset(eps_t[:], 1e-5)

            # weights resident
            encw1b_t = wp.tile([4, HID], bf)
            encw2_t = wp.tile([HID, F], bf)
            encb2_t = wp.tile([F, 1], fp)
            nc.sync.dma_start(out=encw1b_t[:], in_=encw1b_d[:, :])
            nc.sync.dma_start(out=encw2_t[:], in_=encw2_d[:, :])
            nc.sync.dma_start(out=encb2_t[:], in_=encb2_d[:, :])
            wrb1_t = [wp.tile([65, F], bf, name=f"wrb1{l}") for l in range(L)]
            wc_t = [wp.tile([65, F], bf, name=f"wc{l}") for l in range(L)]
            web_t = [wp.tile([2, F], bf, name=f"web{l}") for l in range(L)]
            w2b_t = [wp.tile([65, F], bf, name=f"w2b{l}") for l in range(L)]
            skb_t = [wp.tile([F, 1], fp, name=f"skb{l}") for l in range(L)]
            skw_t = [wp.tile([F, F], bf, name=f"skw{l}") for l in range(L)]
            skwb_t = [wp.tile([65, F], bf, name=f"skwb{l}") for l in range(L)]
            lng_t = [wp.tile([F, 1], fp, name=f"lng{l}") for l in range(L)]
            lnb_t = [wp.tile([F, 1], fp, name=f"lnb{l}") for l in range(L)]
            for l in range(L):
                nc.sync.dma_start(out=wrb1_t[l][:], in_=wrb1_d[l, :, :])
                nc.sync.dma_start(out=wc_t[l][:], in_=wc_d[l, :, :])
                nc.sync.dma_start(out=web_t[l][:], in_=web_d[l, :, :])
                nc.sync.dma_start(out=w2b_t[l][:], in_=w2b_d[l, :, :])
                nc.sync.dma_start(out=skb_t[l][:], in_=skb_d[l, :, :])
                nc.sync.dma_start(out=skw_t[l][:], in_=skw_d[l, :, :])
                nc.sync.dma_start(out=skwb_t[l][:], in_=skwb_d[l, :, :])
                nc.sync.dma_start(out=lng_t[l][:], in_=lng_d[l, :, :])
                nc.sync.dma_start(out=lnb_t[l][:], in_=lnb_d[l, :, :])
            hw1_t = wp.tile([F, F], bf)
            hb1_t = wp.tile([F, 1], fp)
            hw2_t = wp.tile([F, POLY], bf)
            hb2_t = wp.tile([POLY, 1], fp)
            nc.sync.dma_start(out=hw1_t[:], in_=hw1_d[:, :])
            nc.sync.dma_start(out=hb1_t[:], in_=hb1_d[:, :])
            nc.sync.dma_start(out=hw2_t[:], in_=hw2_d[:, :])
            nc.sync.dma_start(out=hb2_t[:], in_=hb2_d[:, :])

            # ---------------- encoder ----------------
            with (
                tc.tile_pool(name="enc_sb", bufs=2) as esb,
                tc.tile_pool(name="enc_nf", bufs=1) as enf,
                tc.tile_pool(name="enc_ps", bufs=2, space="PSUM") as eps,
            ):
                nf_t = enf.tile([4, NSH], bf)
                nc.sync.dma_start(out=nf_t[:], in_=nf1T_d[:, :])
                for (t0, tw) in ntiles:
                    hid_ps = eps.tile([HID, TN], fp, tag="hid")
                    nc.tensor.matmul(out=hid_ps[:, :tw], lhsT=encw1b_t[:],
                                     rhs=nf_t[:, t0:t0 + tw], start=True, stop=True)
                    hid_sb = esb.tile([HID, TN], bf, tag="hsb")
                    nc.vector.tensor_scalar(out=hid_sb[:, :tw], in0=hid_ps[:, :tw],
                                            scalar1=0.0, scalar2=None, op0=AOT.max)
                    h_ps = eps.tile([F, TN], fp, tag="hps")
                    nc.tensor.matmul(out=h_ps[:, :tw], lhsT=encw2_t[:],
                                     rhs=hid_sb[:, :tw], start=True, stop=True)
                    nc.vector.tensor_scalar(out=hT[0:F, t0:t0 + tw], in0=h_ps[:, :tw],
                                            scalar1=encb2_t[:, 0:1], scalar2=None,
                                            op0=AOT.add)

            # ---------------- layers ----------------
            for l in range(L):
                # Q compute per group -> q_local -> AllGather; P overlaps
                with (
                    tc.tile_pool(name=f"pq_sb{l}", bufs=3) as qsb,
                    tc.tile_pool(name=f"pq_ps{l}", bufs=3, space="PSUM") as qps,
                ):
                    for g8 in range(0, G, 8):
                        w = min(8, G - g8)
                        q8_ps = qps.tile([GW, 512], fp, tag="qps")
                        for j in range(w):
                            g = g8 + j
                            nc.tensor.matmul(out=q8_ps[:, j * F:(j + 1) * F],
                                             lhsT=hT[:, g * GW:(g + 1) * GW],
                                             rhs=wc_t[l][:], start=True, stop=True)
                        q8_sb = qsb.tile([GW, 512], bf, tag="qsb")
                        nc.vector.tensor_copy(out=q8_sb[:, 0:w * F],
                                              in_=q8_ps[:, 0:w * F])
                        for j in range(w):
                            g = g8 + j
                            nc.sync.dma_start(
                                out=q_local[g * (GW // 2):(g + 1) * (GW // 2), :],
                                in_=q8_sb[:, j * F:(j + 1) * F])

                    nc.gpsimd.collective_compute(
                        "AllGather", AOT.bypass,
                        replica_groups=[list(range(NCORES))],
                        ins=[q_local[:, :]], outs=[q_full[:, :]],
                    )

                    # P tables (overlap the collective)
                    nc.sync.dma_start(out=PW_all[GW:128, :],
                                      in_=webrep_d[l, :, :])
                    for g8 in range(0, G, 8):
                        w = min(8, G - g8)
                        p8_ps = qps.tile([GW, 512], fp, tag="pps")
                        for j in range(w):
                            g = g8 + j
                            nc.tensor.matmul(out=p8_ps[:, j * F:(j + 1) * F],
                                             lhsT=hT[:, g * GW:(g + 1) * GW],
                                             rhs=wrb1_t[l][:], start=True, stop=True)
                        nc.vector.tensor_copy(
                            out=PW_all[0:GW, g8 * F:(g8 + w) * F],
                            in_=p8_ps[:, 0:w * F])

                # edge phase
                with (
                    tc.tile_pool(name=f"nu_sb{l}", bufs=2) as nsb,
                    tc.tile_pool(name=f"nu_ps{l}", bufs=2, space="PSUM") as nps,
                    tc.tile_pool(name=f"nu_psT{l}", bufs=2, space="PSUM") as npsT,
                    tc.tile_pool(name=f"eg_ix{l}", bufs=6) as gsb,
                    tc.tile_pool(name=f"eg_qg{l}", bufs=6) as qgp,
                    tc.tile_pool(name=f"eg_oh{l}", bufs=3) as ohp,
                    tc.tile_pool(name=f"eg_ohn{l}", bufs=3) as ohnp,
                    tc.tile_pool(name=f"eg_msg{l}", bufs=3) as msb,
                    tc.tile_pool(name=f"eg_ps{l}", bufs=2, space="PSUM") as zps,
                    tc.tile_pool(name=f"agg_ps{l}", bufs=2, space="PSUM") as aps,
                ):
                    def node_tile(t0, tw):
                        sl = slice(t0, t0 + tw)
                        x_ps = nps.tile([128, F], fp, tag="hn")
                        nc.tensor.matmul(out=x_ps[:tw, :], lhsT=aggT[:, sl],
                                         rhs=w2b_t[l][:], start=True, stop=False)
                        nc.tensor.matmul(out=x_ps[:tw, :], lhsT=hT[:, sl],
                                         rhs=skwb_t[l][:], start=False, stop=True)
                        xsq = nsb.tile([128, F], bf, tag="xsq")
                        nc.scalar.activation(out=xsq[:tw, :], in_=x_ps[:tw, :],
                                             func=ACT.Square)
                        s1 = nsb.tile([128, 1], fp, tag="s1")
                        s2 = nsb.tile([128, 1], fp, tag="s2")
                        nc.vector.tensor_reduce(out=s1[:tw], in_=x_ps[:tw, :],
                                                axis=mybir.AxisListType.X,
                                                op=AOT.add)
                        nc.vector.tensor_reduce(out=s2[:tw], in_=xsq[:tw, :],
                                                axis=mybir.AxisListType.X,
                                                op=AOT.add)
                        negmu = nsb.tile([128, 1], fp, tag="negmu")
                        var = nsb.tile([128, 1], fp, tag="var")
                        tmp = nsb.tile([128, 1], fp, tag="tmp")
                        nc.vector.tensor_scalar(out=negmu[:tw], in0=s1[:tw],
                                                scalar1=-1.0 / F, scalar2=None,
                                                op0=AOT.mult)
                        nc.vector.scalar_tensor_tensor(
                            out=tmp[:tw], in0=negmu[:tw], scalar=-1.0,
                            in1=negmu[:tw], op0=AOT.mult, op1=AOT.mult)
                        nc.vector.tensor_scalar(out=var[:tw], in0=s2[:tw],
                                                scalar1=1.0 / F, scalar2=1e-5,
                                                op0=AOT.mult, op1=AOT.add)
                        nc.vector.tensor_tensor(out=var[:tw], in0=var[:tw],
                                                in1=tmp[:tw], op=AOT.add)
                        nc.scalar.activation(out=var[:tw], in_=var[:tw],
                                             func=ACT.Sqrt)
                        nc.vector.reciprocal(out=var[:tw], in_=var[:tw])
                        xn = nsb.tile([128, F], bf, tag="xn")
                        nc.vector.tensor_scalar(out=xn[:tw, :], in0=x_ps[:tw, :],
                                                scalar1=negmu[:tw, 0:1],
                                                scalar2=var[:tw, 0:1],
                                                op0=AOT.add, op1=AOT.mult)
                        yT_ps = npsT.tile([F, 128], bf, tag="yT")
                        nc.tensor.transpose(out=yT_ps[:, :tw], in_=xn[:tw, :],
                                            identity=ident_t[0:tw, 0:tw])
                        nc.scalar.activation(out=hT[0:F, sl], in_=yT_ps[:, :tw],
                                             func=ACT.Relu,
                                             bias=lnb_t[l][:, 0:1],
                                             scale=lng_t[l][:, 0:1])

                    next_tile = 0
                    for bi, b in enumerate(batches):
                        kb, keb, s = b["kb"], b["keb"], b["c0"]
                        cgrp = b["cgrp"]
                        qidx_t = gsb.tile([128, kb * 8], dt.int16, tag="qidx")
                        rloc_t = gsb.tile([128, kb], bf, tag="rloc")
                        nc.sync.dma_start(out=qidx_t[:, :],
                                          in_=qidx_d[:, s * 8:(s + kb) * 8])
                        nc.sync.dma_start(out=rloc_t[:, :], in_=rloc_d[:, s:s + kb])

                        # bf16 pair gather: one call per batch, rotating queues
                        qg = qgp.tile([128, kb, 2 * F], bf, tag="qg")
                        nc.gpsimd.dma_gather(
                            out_ap=qg[:], in_ap=q_full[:, :], idxs_ap=qidx_t[:],
                            num_idxs=kb * 128, num_idxs_reg=kb * 128,
                            elem_size=2 * F, single_packet=False,
                            queue_num=bi % 4)

                        # combined [one-hot; ef] tile: one-hot from int8 rep
                        ohnef_t = ohnp.tile([128, kb * 128], bf, tag="ohnef")
                        rep_t = gsb.tile([GW, kb * 128], dt.int8, tag="rep8")
                        nc.scalar.dma_start(out=rep_t[:, :],
                                            in_=rep8_d[:, s * 128:(s + kb) * 128])
                        nc.scalar.dma_start(out=ohnef_t[GW:128, :],
                                            in_=ef_d[:, s * 128:(s + kb) * 128])
                        nc.vector.tensor_tensor(
                            out=ohnef_t[0:GW, :],
                            in0=rep_t[:, :],
                            in1=iotap8_t[0:GW, 0:1].to_broadcast([GW, kb * 128]),
                            op=AOT.is_equal)

                        # one-hot (edge-partition) for scatter
                        oh_t = ohp.tile([128, kb, GW], bf, tag="oh")
                        nc.vector.tensor_tensor(
                            out=oh_t[:],
                            in0=rloc_t[:, :, None].to_broadcast([128, kb, GW]),
                            in1=iota_t[:, None, :].to_broadcast([128, kb, GW]),
                            op=AOT.is_equal)

                        # pg + ef in one matmul per chunk; msg per slab of 8
                        msg_t = msb.tile([128, kb, F + 1], bf, tag="msg")
                        nc.vector.memset(msg_t[:, :, F:F + 1], 1.0)
                        for (c0, c1, qoff) in ((0, keb, 0), (keb, kb, F)):
                            for s8 in range(c0, c1, 8):
                                w8 = min(8, c1 - s8)
                                pgz_ps = zps.tile([128, 512], fp, tag="z")
                                for j in range(w8):
                                    c = s8 + j
                                    g = int(cgrp[c])
                                    nc.tensor.matmul(
                                        out=pgz_ps[:, j * F:(j + 1) * F],
                                        lhsT=ohnef_t[:, c * 128:(c + 1) * 128],
                                        rhs=PW_all[:, g * F:(g + 1) * F],
                                        start=True, stop=True)
                                nc.vector.tensor_tensor(
                                    out=msg_t[:, s8:s8 + w8, 0:F],
                                    in0=pgz_ps[:, 0:w8 * F].rearrange(
                                        "p (c f) -> p c f", f=F),
                                    in1=qg[:, s8:s8 + w8, qoff:qoff + F],
                                    op=AOT.add)
                                nc.scalar.activation(
                                    out=msg_t[:, s8:s8 + w8, 0:F],
                                    in_=msg_t[:, s8:s8 + w8, 0:F], func=ACT.Relu)

                        # scatter per group
                        for g in b["groups"]:
                            kg = int(K[g])
                            if kg == 0:
                                continue
                            ranges = [b["epos"][g], b["opos"][g]]
                            ranges = [(a, z) for (a, z) in ranges if z > a]
                            agg_ps = aps.tile([F + 1, GW], fp, tag="agg")
                            ci = 0
                            for (a, z) in ranges:
                                for c in range(a, z):
                                    nc.tensor.matmul(
                                        out=agg_ps[:],
                                        lhsT=msg_t[:, c, :],
                                        rhs=oh_t[:, c, :],
                                        start=(ci == 0), stop=(ci == kg - 1))
                                    ci += 1
                            nc.vector.tensor_copy(
                                out=aggT[0:F + 1, g * GW:(g + 1) * GW],
                                in_=agg_ps[:])

                        gdone = min((bi + 1) * GB, G) - 1
                        while next_tile < len(n128):
                            t0, tw = n128[next_tile]
                            gneed = (t0 + tw + GW - 1) // GW - 1
                            if gneed > gdone:
                                break
                            node_tile(t0, tw)
                            next_tile += 1
                    assert next_tile == len(n128)

            # ---------------- head ----------------
            with (
                tc.tile_pool(name="hd_sb", bufs=2) as hsb,
                tc.tile_pool(name="hd_ps", bufs=2, space="PSUM") as hps,
            ):
                for (t0, tw) in ntiles:
                    sl = slice(t0, t0 + tw)
                    z_ps = hps.tile([F, TN], fp, tag="z1")
                    nc.tensor.matmul(out=z_ps[:, :tw], lhsT=hw1_t[:],
                                     rhs=hT[0:F, sl], start=True, stop=True)
                    z_sb = hsb.tile([F, TN], bf, tag="z1sb")
                    nc.vector.tensor_scalar(out=z_sb[:, :tw], in0=z_ps[:, :tw],
                                            scalar1=hb1_t[:, 0:1], scalar2=0.0,
                                            op0=AOT.add, op1=AOT.max)
                    o_ps = hps.tile([POLY, TN], fp, tag="ops")
                    nc.tensor.matmul(out=o_ps[:, :tw], lhsT=hw2_t[:],
                                     rhs=z_sb[:, :tw], start=True, stop=True)
                    o_sb = hsb.tile([POLY, TN], fp, tag="osb")
                    nc.vector.tensor_scalar(out=o_sb[:, :tw], in0=o_ps[:, :tw],
                                            scalar1=hb2_t[:, 0:1], scalar2=None,
                                            op0=AOT.add)
                    nc.sync.dma_start(out=outT_d[:, t0:t0 + tw], in_=o_sb[:, :tw])

    nc.compile()
    return nc


def _run(inputs, trace=False):
    from concourse import bass_utils

    node_features = np.asarray(inputs["node_features"], np.float32)
    edge_index = np.asarray(inputs["edge_index"])
    edge_features = np.asarray(inputs["edge_features"], np.float32)

    sched, percore = _preprocess(node_features, edge_index, edge_features)
    nc = _build(sched)

    # ---- weights (host prep) ----
    s = np.float32
    enc_w1 = np.asarray(inputs["enc_w1"], s)
    enc_b1 = np.asarray(inputs["enc_b1"], s)
    enc_w2 = np.asarray(inputs["enc_w2"], s)
    enc_b2 = np.asarray(inputs["enc_b2"], s)
    conv_w1 = np.asarray(inputs["conv_w1"], s)
    conv_b1 = np.asarray(inputs["conv_b1"], s)
    conv_w2 = np.asarray(inputs["conv_w2"], s)
    conv_b2 = np.asarray(inputs["conv_b2"], s)
    skip_w = np.asarray(inputs["skip_w"], s)
    skip_b = np.asarray(inputs["skip_b"], s)
    ln_g = np.asarray(inputs["ln_g"], s)
    ln_b = np.asarray(inputs["ln_b"], s)
    head_w1 = np.asarray(inputs["head_w1"], s)
    head_b1 = np.asarray(inputs["head_b1"], s)
    head_w2 = np.asarray(inputs["head_w2"], s)
    head_b2 = np.asarray(inputs["head_b2"], s)

    encw1b = np.concatenate([enc_w1, enc_b1[None, :]], axis=0)
    wrb1 = np.concatenate([conv_w1[:, 0:F, :], conv_b1[:, None, :]], axis=1)
    wc = np.concatenate([conv_w1[:, F:2 * F, :],
                         np.zeros((L, 1, F), s)], axis=1)
    web = conv_w1[:, 2 * F:2 * F + 2, :]
    w2b = np.concatenate([conv_w2, conv_b2[:, None, :]], axis=1)

    skwb = np.concatenate([skip_w, skip_b[:, None, :]], axis=1)
    ident = np.eye(128, dtype=s)
    iota = np.tile(np.arange(GW, dtype=s), (128, 1))
    iotap = np.arange(128, dtype=s).reshape(128, 1)
    onesbd = np.zeros((128, 2), s)
    onesbd[0:F, 0] = 1.0 / F
    onesbd[F:2 * F, 1] = 1.0 / F
    ones64 = np.ones((1, F), s)

    b = lambda a: np.ascontiguousarray(a.astype(BF16))
    shared = dict(
        iota=b(iota), iotap=b(iotap),
        iotap8=iotap.astype(np.int8), onesbd=b(onesbd), ones64=b(ones64),
        encw1b=b(encw1b), encw2=b(enc_w2), encb2=enc_b2.reshape(F, 1),
        wrb1=b(wrb1), wc=b(wc), skwb=b(skwb), ident=b(ident), web=b(web),
        webrep=b(np.tile(web, (1, 1, G)).reshape(L, 2, G * F)),
        w2b=b(w2b), skw=b(skip_w),
        skb=skip_b.reshape(L, F, 1),
        lng=ln_g.reshape(L, F, 1), lnb=ln_b.reshape(L, F, 1),
        hw1=b(head_w1), hb1=head_b1.reshape(F, 1),
        hw2=b(head_w2), hb2=head_b2.reshape(POLY, 1),
    )
    in_maps = []
    for c in range(NCORES):
        m = dict(shared)
        m["nf1T"] = percore["nf1T"][c]
        m["qidx"] = percore["qidx_w"][c]
        m["rloc"] = percore["rloc"][c]
        m["rep8"] = percore["rep8"][c]
        m["ef"] = percore["ef"][c]
        in_maps.append(m)

    res = bass_utils.run_bass_kernel_spmd(
        nc, in_maps, core_ids=list(range(NCORES)), trace=trace)
    outs = res.results
    full = np.concatenate([outs[c]["outT"].T for c in range(NCORES)], axis=0)
    return full[:N].astype(np.float32), res


def kernel(**inputs) -> np.ndarray:
    out, _ = _run(inputs, trace=False)
    return out


# revision 26
# speedup vs baseline: 1.0539x; 1.0539x over previous
"""PolyMPNN Trainium2 kernel v3: 4-layer edge-MLP message passing GNN.

Strategy (8 NeuronCores, SPMD single program):
- Nodes sharded contiguously: 6300/core (50400 padded). Each core owns the
  edges whose destination (row) falls in its shard, grouped by 126-node
  windows, split by col parity, padded to 128-edge chunks; chunk schedule
  uniform across cores.
- Per layer: Q = h@W_c computed per group (bf16) -> AllGather; P = h@W_r + b1
  computed per group into a [128, F] table whose rows 126:128 hold the
  edge-feature weights W_e, overlapping the collective.
- Q values fetched per edge with dma_gather over 4 SWDGE queues from the
  bf16 pair table [25200, 128] (256B packets, idx = col>>1, parity picks
  the 64-wide half).
- P + ef term in ONE matmul per chunk: lhsT is a [128, 128e] tile whose rows
  0:126 are the node one-hot (built by a single in-place is_equal over a
  host-replicated rloc block) and rows 126:128 are the edge features.
- Scatter-add by one-hot matmul (agg[65,126] += msg[128e,65].T @ oh_e);
  row 64 (ones col) yields per-node degree for the b2 term.
- Node update: h' = relu(LN(agg@W2 + deg*b2 + skip_b + h@skip_w)), LN in
  feature-on-partition layout using ones-matmul statistics. All matmuls bf16.
"""
import sys

if "/opt/trn_rl_repo" not in sys.path:
    sys.path.insert(0, "/opt/trn_rl_repo")

import numpy as np
import ml_dtypes

BF16 = ml_dtypes.bfloat16

NCORES = 8
N = 50000
NSH = 6300            # nodes per core (= GW * G)
NPAD = NSH * NCORES   # 50400
NPAIR = NPAD // 2     # 25200 node pairs
GW = 126              # node group width (126 + 2 ef rows = 128)
G = NSH // GW         # 50 groups per core
F = 64                # embed
HID = 128             # encoder hidden
L = 4
POLY = 8
TN = 450              # node tile width for matmul passes (14 tiles)
GB = 2                # groups per batch
PH = NSH // 2         # 3150 pairs per core
PH0 = 1600            # pairs in half 0 (node tiles 0..24)
PH1 = PH - PH0        # 1550 pairs in half 1
H0TOT = NCORES * PH0  # 12800


def _wrap_idx(idx_flat: np.ndarray) -> np.ndarray:
    """[n] -> [128, n//16] int16 wrapped (16-lane) + replicated layout."""
    n = len(idx_flat)
    assert n % 16 == 0
    a = idx_flat.reshape(n // 16, 16).T.astype(np.int16)
    return np.ascontiguousarray(np.tile(a, (8, 1)))


def _preprocess(node_features, edge_index, edge_features):
    """Sort/pad edges; build per-core device arrays + shared chunk schedule."""
    rows = edge_index[0].astype(np.int64)
    cols = edge_index[1].astype(np.int64)

    owner = rows // NSH
    lrow = rows % NSH
    grp = lrow // GW
    par = cols & 1

    counts = np.zeros((NCORES, G, 2), np.int64)
    np.add.at(counts, (owner, grp, par), 1)
    Kev = np.ceil(counts[:, :, 0].max(axis=0) / 128).astype(np.int64)
    Kod = np.ceil(counts[:, :, 1].max(axis=0) / 128).astype(np.int64)
    K = Kev + Kod
    C = int(K.sum())

    batches = []
    c0 = 0
    for b0 in range(0, G, GB):
        gs = list(range(b0, min(b0 + GB, G)))
        keb = int(Kev[gs].sum())
        kb = int(K[gs].sum())
        epos, opos = {}, {}
        e_off, o_off = 0, keb
        for g in gs:
            epos[g] = (e_off, e_off + int(Kev[g]))
            opos[g] = (o_off, o_off + int(Kod[g]))
            e_off += int(Kev[g])
            o_off += int(Kod[g])
        cgrp = np.zeros(kb, np.int64)
        for g in gs:
            cgrp[epos[g][0]:epos[g][1]] = g
            cgrp[opos[g][0]:opos[g][1]] = g
        batches.append(dict(groups=gs, c0=c0, kb=kb, keb=keb,
                            epos=epos, opos=opos, cgrp=cgrp))
        c0 += kb
    assert c0 == C

    order = np.lexsort((par, grp, owner))
    srows, scols, sgrp, sowner, spar = (lrow[order], cols[order], grp[order],
                                        owner[order], par[order])
    sef = edge_features[order].astype(np.float32)

    slot_base = np.zeros((NCORES, G, 2), np.int64)
    for b in batches:
        for g in b["groups"]:
            slot_base[:, g, 0] = (b["c0"] + b["epos"][g][0]) * 128
            slot_base[:, g, 1] = (b["c0"] + b["opos"][g][0]) * 128

    key = (sowner * G + sgrp) * 2 + spar
    _, first_idx, key_counts = np.unique(key, return_index=True,
                                         return_counts=True)
    rank = np.arange(len(key), dtype=np.int64)
    rank -= np.repeat(first_idx, key_counts)
    slot = slot_base[sowner, sgrp, spar] + rank

    qidx = np.zeros((NCORES, C * 128), np.int64)
    rloc = np.full((NCORES, 128, C), 999.0, np.float32)
    rflat = np.full((NCORES, C * 128), 999.0, np.float32)
    ef = np.zeros((NCORES, 2, C * 128), np.float32)
    qidx[sowner, slot] = scols >> 1
    lane = slot % 128
    chunk = slot // 128
    rloc[sowner, lane, chunk] = (srows % GW).astype(np.float32)
    rflat[sowner, slot] = (srows % GW).astype(np.float32)
    ef[sowner, 0, slot] = sef[:, 0]
    ef[sowner, 1, slot] = sef[:, 1]


    # rep8: replicated rloc rows (slot-major) in int8; sentinel 127
    r8 = np.where(rflat >= GW, 127, rflat).astype(np.int8)
    rep8 = np.broadcast_to(r8[:, None, :], (NCORES, GW, C * 128)).copy()

    qidx_w = np.zeros((NCORES, 128, C * 8), np.int16)
    for c in range(NCORES):
        for b in batches:
            s, kb = b["c0"], b["kb"]
            qidx_w[c][:, s * 8:(s + kb) * 8] = _wrap_idx(
                qidx[c][s * 128:(s + kb) * 128])

    nf = np.zeros((NPAD, 3), np.float32)
    nf[:N] = node_features
    nf1T = np.zeros((NCORES, 4, NSH), np.float32)
    for c in range(NCORES):
        nf1T[c, 0:3] = nf[c * NSH:(c + 1) * NSH].T
        nf1T[c, 3] = 1.0

    sched = dict(K=K, C=C, batches=batches)
    percore = dict(qidx_w=qidx_w,
                   rloc=rloc.astype(BF16),
                   rep8=rep8,
                   ef=ef.astype(BF16),
                   nf1T=nf1T.astype(BF16))
    return sched, percore


def _build(sched):
    """Build the Bass program for the shared chunk schedule."""
    import concourse.mybir as mybir
    import concourse.tile as tile
    from concourse import bacc

    dt = mybir.dt
    fp = dt.float32
    bf = dt.bfloat16
    AOT = mybir.AluOpType
    ACT = mybir.ActivationFunctionType

    C = sched["C"]
    batches = sched["batches"]
    K = sched["K"]

    nc = bacc.Bacc("TRN2", num_devices=NCORES, num_swdge_queues=4)

    # ---- I/O ----
    nf1T_d = nc.dram_tensor("nf1T", [4, NSH], bf, kind="ExternalInput")
    qidx_d = nc.dram_tensor("qidx", [128, C * 8], dt.int16, kind="ExternalInput")
    rloc_d = nc.dram_tensor("rloc", [128, C], bf, kind="ExternalInput")
    rep8_d = nc.dram_tensor("rep8", [GW, C * 128], dt.int8, kind="ExternalInput")
    ef_d = nc.dram_tensor("ef", [2, C * 128], bf, kind="ExternalInput")
    iotap8_d = nc.dram_tensor("iotap8", [128, 1], dt.int8, kind="ExternalInput")
    iota_d = nc.dram_tensor("iota", [128, GW], bf, kind="ExternalInput")
    iotap_d = nc.dram_tensor("iotap", [128, 1], bf, kind="ExternalInput")
    onesbd_d = nc.dram_tensor("onesbd", [128, 2], bf, kind="ExternalInput")
    ones64_d = nc.dram_tensor("ones64", [1, 64], bf, kind="ExternalInput")
    encw1b_d = nc.dram_tensor("encw1b", [4, HID], bf, kind="ExternalInput")
    encw2_d = nc.dram_tensor("encw2", [HID, F], bf, kind="ExternalInput")
    encb2_d = nc.dram_tensor("encb2", [F, 1], fp, kind="ExternalInput")
    wrb1_d = nc.dram_tensor("wrb1", [L, 65, F], bf, kind="ExternalInput")
    wc_d = nc.dram_tensor("wc", [L, 65, F], bf, kind="ExternalInput")
    web_d = nc.dram_tensor("web", [L, 2, F], bf, kind="ExternalInput")
    webrep_d = nc.dram_tensor("webrep", [L, 2, G * F], bf, kind="ExternalInput")
    w2b_d = nc.dram_tensor("w2b", [L, 65, F], bf, kind="ExternalInput")
    skb_d = nc.dram_tensor("skb", [L, F, 1], fp, kind="ExternalInput")
    skw_d = nc.dram_tensor("skw", [L, F, F], bf, kind="ExternalInput")
    skwb_d = nc.dram_tensor("skwb", [L, 65, F], bf, kind="ExternalInput")
    ident_d = nc.dram_tensor("ident", [128, 128], bf, kind="ExternalInput")
    lng_d = nc.dram_tensor("lng", [L, F, 1], fp, kind="ExternalInput")
    lnb_d = nc.dram_tensor("lnb", [L, F, 1], fp, kind="ExternalInput")
    hw1_d = nc.dram_tensor("hw1", [F, F], bf, kind="ExternalInput")
    hb1_d = nc.dram_tensor("hb1", [F, 1], fp, kind="ExternalInput")
    hw2_d = nc.dram_tensor("hw2", [F, POLY], bf, kind="ExternalInput")
    hb2_d = nc.dram_tensor("hb2", [POLY, 1], fp, kind="ExternalInput")
    outT_d = nc.dram_tensor("outT", [POLY, NSH], fp, kind="ExternalOutput")
    # internal (bf16 pair layout: row j holds nodes 2j, 2j+1)
    q_local = nc.dram_tensor("q_local", [NSH // 2, 2 * F], bf)
    q_full = nc.dram_tensor("q_full", [NPAIR, 2 * F], bf, addr_space="Shared")

    ntiles = [(t * TN, min(TN, NSH - t * TN)) for t in range((NSH + TN - 1) // TN)]
    n128 = [(t * 128, min(128, NSH - t * 128)) for t in range((NSH + 127) // 128)]

    with tile.TileContext(nc) as tc:
        with (
            tc.tile_pool(name="persist", bufs=1) as pp,
            tc.tile_pool(name="wts", bufs=1) as wp,
        ):
            # persistent state
            hT = pp.tile([65, NSH], bf)         # rows 0-63 h, row 64 ones
            aggT = pp.tile([65, NSH], bf)       # rows 0-63 agg, row 64 deg
            PW_all = pp.tile([128, G * F], bf)  # rows 0:126 P_g, 126:128 W_e
            iota_t = pp.tile([128, GW], bf)
            iotap_t = pp.tile([128, 1], bf)
            iotap8_t = pp.tile([128, 1], dt.int8)
            nc.sync.dma_start(out=iotap8_t[:], in_=iotap8_d[:, :])
            ident_t = pp.tile([128, 128], bf)
            nc.sync.dma_start(out=ident_t[:], in_=ident_d[:, :])
            onesbd_t = pp.tile([128, 2], bf)
            ones64_t = pp.tile([1, 64], bf)
            nc.sync.dma_start(out=iota_t[:], in_=iota_d[:, :])
            nc.sync.dma_start(out=iotap_t[:], in_=iotap_d[:, :])
            nc.sync.dma_start(out=onesbd_t[:], in_=onesbd_d[:, :])
            nc.sync.dma_start(out=ones64_t[:], in_=ones64_d[:, :])
            nc.vector.memset(hT[64:65, :], 1.0)
            eps_t = pp.tile([1, 1], fp)
            nc.vector.memset(eps_t[:], 1e-5)

            # weights resident
            encw1b_t = wp.tile([4, HID], bf)
            encw2_t = wp.tile([HID, F], bf)
            encb2_t = wp.tile([F, 1], fp)
            nc.sync.dma_start(out=encw1b_t[:], in_=encw1b_d[:, :])
            nc.sync.dma_start(out=encw2_t[:], in_=encw2_d[:, :])
            nc.sync.dma_start(out=encb2_t[:], in_=encb2_d[:, :])
            wrb1_t = [wp.tile([65, F], bf, name=f"wrb1{l}") for l in range(L)]
            wc_t = [wp.tile([65, F], bf, name=f"wc{l}") for l in range(L)]
            web_t = [wp.tile([2, F], bf, name=f"web{l}") for l in range(L)]
            w2b_t = [wp.tile([65, F], bf, name=f"w2b{l}") for l in range(L)]
            skb_t = [wp.tile([F, 1], fp, name=f"skb{l}") for l in range(L)]
            skw_t = [wp.tile([F, F], bf, name=f"skw{l}") for l in range(L)]
            skwb_t = [wp.tile([65, F], bf, name=f"skwb{l}") for l in range(L)]
            lng_t = [wp.tile([F, 1], fp, name=f"lng{l}") for l in range(L)]
            lnb_t = [wp.tile([F, 1], fp, name=f"lnb{l}") for l in range(L)]
            for l in range(L):
                nc.sync.dma_start(out=wrb1_t[l][:], in_=wrb1_d[l, :, :])
                nc.sync.dma_start(out=wc_t[l][:], in_=wc_d[l, :, :])
                nc.sync.dma_start(out=web_t[l][:], in_=web_d[l, :, :])
                nc.sync.dma_start(out=w2b_t[l][:], in_=w2b_d[l, :, :])
                nc.sync.dma_start(out=skb_t[l][:], in_=skb_d[l, :, :])
                nc.sync.dma_start(out=skw_t[l][:], in_=skw_d[l, :, :])
                nc.sync.dma_start(out=skwb_t[l][:], in_=skwb_d[l, :, :])
                nc.sync.dma_start(out=lng_t[l][:], in_=lng_d[l, :, :])
                nc.sync.dma_start(out=lnb_t[l][:], in_=lnb_d[l, :, :])
            hw1_t = wp.tile([F, F], bf)
            hb1_t = wp.tile([F, 1], fp)
            hw2_t = wp.tile([F, POLY], bf)
            hb2_t = wp.tile([POLY, 1], fp)
            nc.sync.dma_start(out=hw1_t[:], in_=hw1_d[:, :])
            nc.sync.dma_start(out=hb1_t[:], in_=hb1_d[:, :])
            nc.sync.dma_start(out=hw2_t[:], in_=hw2_d[:, :])
            nc.sync.dma_start(out=hb2_t[:], in_=hb2_d[:, :])

            # ---------------- encoder ----------------
            with (
                tc.tile_pool(name="enc_sb", bufs=2) as esb,
                tc.tile_pool(name="enc_nf", bufs=1) as enf,
                tc.tile_pool(name="enc_ps", bufs=2, space="PSUM") as eps,
            ):
                nf_t = enf.tile([4, NSH], bf)
                nc.sync.dma_start(out=nf_t[:], in_=nf1T_d[:, :])
                for (t0, tw) in ntiles:
                    hid_ps = eps.tile([HID, TN], fp, tag="hid")
                    nc.tensor.matmul(out=hid_ps[:, :tw], lhsT=encw1b_t[:],
                                     rhs=nf_t[:, t0:t0 + tw], start=True, stop=True)
                    hid_sb = esb.tile([HID, TN], bf, tag="hsb")
                    nc.vector.tensor_scalar(out=hid_sb[:, :tw], in0=hid_ps[:, :tw],
                                            scalar1=0.0, scalar2=None, op0=AOT.max)
                    h_ps = eps.tile([F, TN], fp, tag="hps")
                    nc.tensor.matmul(out=h_ps[:, :tw], lhsT=encw2_t[:],
                                     rhs=hid_sb[:, :tw], start=True, stop=True)
                    nc.vector.tensor_scalar(out=hT[0:F, t0:t0 + tw], in0=h_ps[:, :tw],
                                            scalar1=encb2_t[:, 0:1], scalar2=None,
                                            op0=AOT.add)

            # ---------------- layers ----------------
            for l in range(L):
                # Q compute per group -> q_local -> AllGather; P overlaps
                with (
                    tc.tile_pool(name=f"pq_sb{l}", bufs=3) as qsb,
                    tc.tile_pool(name=f"pq_ps{l}", bufs=3, space="PSUM") as qps,
                ):
                    for g8 in range(0, G, 8):
                        w = min(8, G - g8)
                        q8_ps = qps.tile([GW, 512], fp, tag="qps")
                        for j in range(w):
                            g = g8 + j
                            nc.tensor.matmul(out=q8_ps[:, j * F:(j + 1) * F],
                                             lhsT=hT[:, g * GW:(g + 1) * GW],
                                             rhs=wc_t[l][:], start=True, stop=True)
                        q8_sb = qsb.tile([GW, 512], bf, tag="qsb")
                        nc.vector.tensor_copy(out=q8_sb[:, 0:w * F],
                                              in_=q8_ps[:, 0:w * F])
                        for j in range(w):
                            g = g8 + j
                            nc.sync.dma_start(
                                out=q_local[g * (GW // 2):(g + 1) * (GW // 2), :],
                                in_=q8_sb[:, j * F:(j + 1) * F])

                    nc.gpsimd.collective_compute(
                        "AllGather", AOT.bypass,
                        replica_groups=[list(range(NCORES))],
                        ins=[q_local[:, :]], outs=[q_full[:, :]],
                    )

                    # P tables (overlap the collective)
                    nc.sync.dma_start(out=PW_all[GW:128, :],
                                      in_=webrep_d[l, :, :])
                    for g8 in range(0, G, 8):
                        w = min(8, G - g8)
                        p8_ps = qps.tile([GW, 512], fp, tag="pps")
                        for j in range(w):
                            g = g8 + j
                            nc.tensor.matmul(out=p8_ps[:, j * F:(j + 1) * F],
                                             lhsT=hT[:, g * GW:(g + 1) * GW],
                                             rhs=wrb1_t[l][:], start=True, stop=True)
                        nc.vector.tensor_copy(
                            out=PW_all[0:GW, g8 * F:(g8 + w) * F],
                            in_=p8_ps[:, 0:w * F])

                # edge phase
                with (
                    tc.tile_pool(name=f"nu_sb{l}", bufs=2) as nsb,
                    tc.tile_pool(name=f"nu_ps{l}", bufs=2, space="PSUM") as nps,
                    tc.tile_pool(name=f"nu_psT{l}", bufs=2, space="PSUM") as npsT,
                    tc.tile_pool(name=f"eg_ix{l}", bufs=6) as gsb,
                    tc.tile_pool(name=f"eg_qg{l}", bufs=6) as qgp,
                    tc.tile_pool(name=f"eg_oh{l}", bufs=3) as ohp,
                    tc.tile_pool(name=f"eg_ohn{l}", bufs=3) as ohnp,
                    tc.tile_pool(name=f"eg_msg{l}", bufs=3) as msb,
                    tc.tile_pool(name=f"eg_ps{l}", bufs=2, space="PSUM") as zps,
                    tc.tile_pool(name=f"agg_ps{l}", bufs=2, space="PSUM") as aps,
                ):
                    def node_tile(t0, tw):
                        sl = slice(t0, t0 + tw)
                        x_ps = nps.tile([128, F], fp, tag="hn")
                        nc.tensor.matmul(out=x_ps[:tw, :], lhsT=aggT[:, sl],
                                         rhs=w2b_t[l][:], start=True, stop=False)
                        nc.tensor.matmul(out=x_ps[:tw, :], lhsT=hT[:, sl],
                                         rhs=skwb_t[l][:], start=False, stop=True)
                        xsq = nsb.tile([128, F], bf, tag="xsq")
                        nc.scalar.activation(out=xsq[:tw, :], in_=x_ps[:tw, :],
                                             func=ACT.Square)
                        s1 = nsb.tile([128, 1], fp, tag="s1")
                        s2 = nsb.tile([128, 1], fp, tag="s2")
                        nc.vector.tensor_reduce(out=s1[:tw], in_=x_ps[:tw, :],
                                                axis=mybir.AxisListType.X,
                                                op=AOT.add)
                        nc.vector.tensor_reduce(out=s2[:tw], in_=xsq[:tw, :],
                                                axis=mybir.AxisListType.X,
                                                op=AOT.add)
                        negmu = nsb.tile([128, 1], fp, tag="negmu")
                        var = nsb.tile([128, 1], fp, tag="var")
                        tmp = nsb.tile([128, 1], fp, tag="tmp")
                        nc.vector.tensor_scalar(out=negmu[:tw], in0=s1[:tw],
                                                scalar1=-1.0 / F, scalar2=None,
                                                op0=AOT.mult)
                        nc.vector.scalar_tensor_tensor(
                            out=tmp[:tw], in0=negmu[:tw], scalar=-1.0,
                            in1=negmu[:tw], op0=AOT.mult, op1=AOT.mult)
                        nc.vector.tensor_scalar(out=var[:tw], in0=s2[:tw],
                                                scalar1=1.0 / F, scalar2=1e-5,
                                                op0=AOT.mult, op1=AOT.add)
                        nc.vector.tensor_tensor(out=var[:tw], in0=var[:tw],
                                                in1=tmp[:tw], op=AOT.add)
                        nc.scalar.activation(out=var[:tw], in_=var[:tw],
                                             func=ACT.Sqrt)
                        nc.vector.reciprocal(out=var[:tw], in_=var[:tw])
                        xn = nsb.tile([128, F], bf, tag="xn")
                        nc.vector.tensor_scalar(out=xn[:tw, :], in0=x_ps[:tw, :],
                                                scalar1=negmu[:tw, 0:1],
                                                scalar2=var[:tw, 0:1],
                                                op0=AOT.add, op1=AOT.mult)
                        yT_ps = npsT.tile([F, 128], bf, tag="yT")
                        nc.tensor.transpose(out=yT_ps[:, :tw], in_=xn[:tw, :],
                                            identity=ident_t[0:tw, 0:tw])
                        nc.scalar.activation(out=hT[0:F, sl], in_=yT_ps[:, :tw],
                                             func=ACT.Relu,
                                             bias=lnb_t[l][:, 0:1],
                                             scale=lng_t[l][:, 0:1])

                    next_tile = 0
                    for bi, b in enumerate(batches):
                        kb, keb, s = b["kb"], b["keb"], b["c0"]
                        cgrp = b["cgrp"]
                        qidx_t = gsb.tile([128, kb * 8], dt.int16, tag="qidx")
                        rloc_t = gsb.tile([128, kb], bf, tag="rloc")
                        nc.sync.dma_start(out=qidx_t[:, :],
                                          in_=qidx_d[:, s * 8:(s + kb) * 8])
                        nc.sync.dma_start(out=rloc_t[:, :], in_=rloc_d[:, s:s + kb])

                        # bf16 pair gather: one call per batch, rotating queues
                        qg = qgp.tile([128, kb, 2 * F], bf, tag="qg")
                        nc.gpsimd.dma_gather(
                            out_ap=qg[:], in_ap=q_full[:, :], idxs_ap=qidx_t[:],
                            num_idxs=kb * 128, num_idxs_reg=kb * 128,
                            elem_size=2 * F, single_packet=False,
                            queue_num=bi % 4)

                        # combined [one-hot; ef] tile: one-hot from int8 rep
                        ohnef_t = ohnp.tile([128, kb * 128], bf, tag="ohnef")
                        rep_t = gsb.tile([GW, kb * 128], dt.int8, tag="rep8")
                        nc.sync.dma_start(out=rep_t[:, :],
                                          in_=rep8_d[:, s * 128:(s + kb) * 128])
                        nc.sync.dma_start(out=ohnef_t[GW:128, :],
                                          in_=ef_d[:, s * 128:(s + kb) * 128])
                        nc.vector.tensor_tensor(
                            out=ohnef_t[0:GW, :],
                            in0=rep_t[:, :],
                            in1=iotap8_t[0:GW, 0:1].to_broadcast([GW, kb * 128]),
                            op=AOT.is_equal)

                        # one-hot (edge-partition) for scatter
                        oh_t = ohp.tile([128, kb, GW], bf, tag="oh")
                        nc.vector.tensor_tensor(
                            out=oh_t[:],
                            in0=rloc_t[:, :, None].to_broadcast([128, kb, GW]),
                            in1=iota_t[:, None, :].to_broadcast([128, kb, GW]),
                            op=AOT.is_equal)

                        # pg + ef in one matmul per chunk; msg per slab of 8
                        msg_t = msb.tile([128, kb, F + 1], bf, tag="msg")
                        nc.vector.memset(msg_t[:, :, F:F + 1], 1.0)
                        for (c0, c1, qoff) in ((0, keb, 0), (keb, kb, F)):
                            for s8 in range(c0, c1, 8):
                                w8 = min(8, c1 - s8)
                                pgz_ps = zps.tile([128, 512], fp, tag="z")
                                for j in range(w8):
                                    c = s8 + j
                                    g = int(cgrp[c])
                                    nc.tensor.matmul(
                                        out=pgz_ps[:, j * F:(j + 1) * F],
                                        lhsT=ohnef_t[:, c * 128:(c + 1) * 128],
                                        rhs=PW_all[:, g * F:(g + 1) * F],
                                        start=True, stop=True)
                                nc.vector.tensor_tensor(
                                    out=msg_t[:, s8:s8 + w8, 0:F],
                                    in0=pgz_ps[:, 0:w8 * F].rearrange(
                                        "p (c f) -> p c f", f=F),
                                    in1=qg[:, s8:s8 + w8, qoff:qoff + F],
                                    op=AOT.add)
                                nc.scalar.activation(
                                    out=msg_t[:, s8:s8 + w8, 0:F],
                                    in_=msg_t[:, s8:s8 + w8, 0:F], func=ACT.Relu)

                        # scatter per group
                        for g in b["groups"]:
                            kg = int(K[g])
                            if kg == 0:
                                continue
                            ranges = [b["epos"][g], b["opos"][g]]
                            ranges = [(a, z) for (a, z) in ranges if z > a]
                            agg_ps = aps.tile([F + 1, GW], fp, tag="agg")
                            ci = 0
                            for (a, z) in ranges:
                                for c in range(a, z):
                                    nc.tensor.matmul(
                                        out=agg_ps[:],
                                        lhsT=msg_t[:, c, :],
                                        rhs=oh_t[:, c, :],
                                        start=(ci == 0), stop=(ci == kg - 1))
                                    ci += 1
                            nc.vector.tensor_copy(
                                out=aggT[0:F + 1, g * GW:(g + 1) * GW],
                                in_=agg_ps[:])

                        gdone = min((bi + 1) * GB, G) - 1
                        while next_tile < len(n128):
                            t0, tw = n128[next_tile]
                            gneed = (t0 + tw + GW - 1) // GW - 1
                            if gneed > gdone:
                                break
                            node_tile(t0, tw)
                            next_tile += 1
                    assert next_tile == len(n128)

            # ---------------- head ----------------
            with (
                tc.tile_pool(name="hd_sb", bufs=2) as hsb,
                tc.tile_pool(name="hd_ps", bufs=2, space="PSUM") as hps,
            ):
                for (t0, tw) in ntiles:
                    sl = slice(t0, t0 + tw)
                    z_ps = hps.tile([F, TN], fp, tag="z1")
                    nc.tensor.matmul(out=z_ps[:, :tw], lhsT=hw1_t[:],
                                     rhs=hT[0:F, sl], start=True, stop=True)
                    z_sb = hsb.tile([F, TN], bf, tag="z1sb")
                    nc.vector.tensor_scalar(out=z_sb[:, :tw], in0=z_ps[:, :tw],
                                            scalar1=hb1_t[:, 0:1], scalar2=0.0,
                                            op0=AOT.add, op1=AOT.max)
                    o_ps = hps.tile([POLY, TN], fp, tag="ops")
                    nc.tensor.matmul(out=o_ps[:, :tw], lhsT=hw2_t[:],
                                     rhs=z_sb[:, :tw], start=True, stop=True)
                    o_sb = hsb.tile([POLY, TN], fp, tag="osb")
                    nc.vector.tensor_scalar(out=o_sb[:, :tw], in0=o_ps[:, :tw],
                                            scalar1=hb2_t[:, 0:1], scalar2=None,
                                            op0=AOT.add)
                    nc.sync.dma_start(out=outT_d[:, t0:t0 + tw], in_=o_sb[:, :tw])

    nc.compile()
    return nc


def _run(inputs, trace=False):
    from concourse import bass_utils

    node_features = np.asarray(inputs["node_features"], np.float32)
    edge_index = np.asarray(inputs["edge_index"])
    edge_features = np.asarray(inputs["edge_features"], np.float32)

    sched, percore = _preprocess(node_features, edge_index, edge_features)
    nc = _build(sched)

    # ---- weights (host prep) ----
    s = np.float32
    enc_w1 = np.asarray(inputs["enc_w1"], s)
    enc_b1 = np.asarray(inputs["enc_b1"], s)
    enc_w2 = np.asarray(inputs["enc_w2"], s)
    enc_b2 = np.asarray(inputs["enc_b2"], s)
    conv_w1 = np.asarray(inputs["conv_w1"], s)
    conv_b1 = np.asarray(inputs["conv_b1"], s)
    conv_w2 = np.asarray(inputs["conv_w2"], s)
    conv_b2 = np.asarray(inputs["conv_b2"], s)
    skip_w = np.asarray(inputs["skip_w"], s)
    skip_b = np.asarray(inputs["skip_b"], s)
    ln_g = np.asarray(inputs["ln_g"], s)
    ln_b = np.asarray(inputs["ln_b"], s)
    head_w1 = np.asarray(inputs["head_w1"], s)
    head_b1 = np.asarray(inputs["head_b1"], s)
    head_w2 = np.asarray(inputs["head_w2"], s)
    head_b2 = np.asarray(inputs["head_b2"], s)

    encw1b = np.concatenate([enc_w1, enc_b1[None, :]], axis=0)
    wrb1 = np.concatenate([conv_w1[:, 0:F, :], conv_b1[:, None, :]], axis=1)
    wc = np.concatenate([conv_w1[:, F:2 * F, :],
                         np.zeros((L, 1, F), s)], axis=1)
    web = conv_w1[:, 2 * F:2 * F + 2, :]
    w2b = np.concatenate([conv_w2, conv_b2[:, None, :]], axis=1)

    skwb = np.concatenate([skip_w, skip_b[:, None, :]], axis=1)
    ident = np.eye(128, dtype=s)
    iota = np.tile(np.arange(GW, dtype=s), (128, 1))
    iotap = np.arange(128, dtype=s).reshape(128, 1)
    onesbd = np.zeros((128, 2), s)
    onesbd[0:F, 0] = 1.0 / F
    onesbd[F:2 * F, 1] = 1.0 / F
    ones64 = np.ones((1, F), s)

    b = lambda a: np.ascontiguousarray(a.astype(BF16))
    shared = dict(
        iota=b(iota), iotap=b(iotap),
        iotap8=iotap.astype(np.int8), onesbd=b(onesbd), ones64=b(ones64),
        encw1b=b(encw1b), encw2=b(enc_w2), encb2=enc_b2.reshape(F, 1),
        wrb1=b(wrb1), wc=b(wc), skwb=b(skwb), ident=b(ident), web=b(web),
        webrep=b(np.tile(web, (1, 1, G)).reshape(L, 2, G * F)),
        w2b=b(w2b), skw=b(skip_w),
        skb=skip_b.reshape(L, F, 1),
        lng=ln_g.reshape(L, F, 1), lnb=ln_b.reshape(L, F, 1),
        hw1=b(head_w1), hb1=head_b1.reshape(F, 1),
        hw2=b(head_w2), hb2=head_b2.reshape(POLY, 1),
    )
    in_maps = []
    for c in range(NCORES):
        m = dict(shared)
        m["nf1T"] = percore["nf1T"][c]
        m["qidx"] = percore["qidx_w"][c]
        m["rloc"] = percore["rloc"][c]
        m["rep8"] = percore["rep8"][c]
        m["ef"] = percore["ef"][c]
        in_maps.append(m)

    res = bass_utils.run_bass_kernel_spmd(
        nc, in_maps, core_ids=list(range(NCORES)), trace=trace)
    outs = res.results
    full = np.concatenate([outs[c]["outT"].T for c in range(NCORES)], axis=0)
    return full[:N].astype(np.float32), res


def kernel(**inputs) -> np.ndarray:
    out, _ = _run(inputs, trace=False)
    return out


# revision 27
# speedup vs baseline: 1.0646x; 1.0101x over previous
"""PolyMPNN Trainium2 kernel v3: 4-layer edge-MLP message passing GNN.

Strategy (8 NeuronCores, SPMD single program):
- Nodes sharded contiguously: 6300/core (50400 padded). Each core owns the
  edges whose destination (row) falls in its shard, grouped by 126-node
  windows, split by col parity, padded to 128-edge chunks; chunk schedule
  uniform across cores.
- Per layer: Q = h@W_c computed per group (bf16) -> AllGather; P = h@W_r + b1
  computed per group into a [128, F] table whose rows 126:128 hold the
  edge-feature weights W_e, overlapping the collective.
- Q values fetched per edge with dma_gather over 4 SWDGE queues from the
  bf16 pair table [25200, 128] (256B packets, idx = col>>1, parity picks
  the 64-wide half).
- P + ef term in ONE matmul per chunk: lhsT is a [128, 128e] tile whose rows
  0:126 are the node one-hot (built by a single in-place is_equal over a
  host-replicated rloc block) and rows 126:128 are the edge features.
- Scatter-add by one-hot matmul (agg[65,126] += msg[128e,65].T @ oh_e);
  row 64 (ones col) yields per-node degree for the b2 term.
- Node update: h' = relu(LN(agg@W2 + deg*b2 + skip_b + h@skip_w)), LN in
  feature-on-partition layout using ones-matmul statistics. All matmuls bf16.
"""
import sys

if "/opt/trn_rl_repo" not in sys.path:
    sys.path.insert(0, "/opt/trn_rl_repo")

import numpy as np
import ml_dtypes

BF16 = ml_dtypes.bfloat16

NCORES = 8
N = 50000
NSH = 6300            # nodes per core (= GW * G)
NPAD = NSH * NCORES   # 50400
NPAIR = NPAD // 2     # 25200 node pairs
GW = 126              # node group width (126 + 2 ef rows = 128)
G = NSH // GW         # 50 groups per core
F = 64                # embed
HID = 128             # encoder hidden
L = 4
POLY = 8
TN = 450              # node tile width for matmul passes (14 tiles)
GB = 1                # groups per batch
PH = NSH // 2         # 3150 pairs per core
PH0 = 1600            # pairs in half 0 (node tiles 0..24)
PH1 = PH - PH0        # 1550 pairs in half 1
H0TOT = NCORES * PH0  # 12800


def _wrap_idx(idx_flat: np.ndarray) -> np.ndarray:
    """[n] -> [128, n//16] int16 wrapped (16-lane) + replicated layout."""
    n = len(idx_flat)
    assert n % 16 == 0
    a = idx_flat.reshape(n // 16, 16).T.astype(np.int16)
    return np.ascontiguousarray(np.tile(a, (8, 1)))


def _preprocess(node_features, edge_index, edge_features):
    """Sort/pad edges; build per-core device arrays + shared chunk schedule."""
    rows = edge_index[0].astype(np.int64)
    cols = edge_index[1].astype(np.int64)

    owner = rows // NSH
    lrow = rows % NSH
    grp = lrow // GW
    par = cols & 1

    counts = np.zeros((NCORES, G, 2), np.int64)
    np.add.at(counts, (owner, grp, par), 1)
    Kev = np.ceil(counts[:, :, 0].max(axis=0) / 128).astype(np.int64)
    Kod = np.ceil(counts[:, :, 1].max(axis=0) / 128).astype(np.int64)
    K = Kev + Kod
    C = int(K.sum())

    batches = []
    c0 = 0
    for b0 in range(0, G, GB):
        gs = list(range(b0, min(b0 + GB, G)))
        keb = int(Kev[gs].sum())
        kb = int(K[gs].sum())
        epos, opos = {}, {}
        e_off, o_off = 0, keb
        for g in gs:
            epos[g] = (e_off, e_off + int(Kev[g]))
            opos[g] = (o_off, o_off + int(Kod[g]))
            e_off += int(Kev[g])
            o_off += int(Kod[g])
        cgrp = np.zeros(kb, np.int64)
        for g in gs:
            cgrp[epos[g][0]:epos[g][1]] = g
            cgrp[opos[g][0]:opos[g][1]] = g
        batches.append(dict(groups=gs, c0=c0, kb=kb, keb=keb,
                            epos=epos, opos=opos, cgrp=cgrp))
        c0 += kb
    assert c0 == C

    order = np.lexsort((par, grp, owner))
    srows, scols, sgrp, sowner, spar = (lrow[order], cols[order], grp[order],
                                        owner[order], par[order])
    sef = edge_features[order].astype(np.float32)

    slot_base = np.zeros((NCORES, G, 2), np.int64)
    for b in batches:
        for g in b["groups"]:
            slot_base[:, g, 0] = (b["c0"] + b["epos"][g][0]) * 128
            slot_base[:, g, 1] = (b["c0"] + b["opos"][g][0]) * 128

    key = (sowner * G + sgrp) * 2 + spar
    _, first_idx, key_counts = np.unique(key, return_index=True,
                                         return_counts=True)
    rank = np.arange(len(key), dtype=np.int64)
    rank -= np.repeat(first_idx, key_counts)
    slot = slot_base[sowner, sgrp, spar] + rank

    qidx = np.zeros((NCORES, C * 128), np.int64)
    rloc = np.full((NCORES, 128, C), 999.0, np.float32)
    rflat = np.full((NCORES, C * 128), 999.0, np.float32)
    ef = np.zeros((NCORES, 2, C * 128), np.float32)
    qidx[sowner, slot] = scols >> 1
    lane = slot % 128
    chunk = slot // 128
    rloc[sowner, lane, chunk] = (srows % GW).astype(np.float32)
    rflat[sowner, slot] = (srows % GW).astype(np.float32)
    ef[sowner, 0, slot] = sef[:, 0]
    ef[sowner, 1, slot] = sef[:, 1]


    # rep8: replicated rloc rows (slot-major) in int8; sentinel 127
    r8 = np.where(rflat >= GW, 127, rflat).astype(np.int8)
    rep8 = np.broadcast_to(r8[:, None, :], (NCORES, GW, C * 128)).copy()

    qidx_w = np.zeros((NCORES, 128, C * 8), np.int16)
    for c in range(NCORES):
        for b in batches:
            s, kb = b["c0"], b["kb"]
            qidx_w[c][:, s * 8:(s + kb) * 8] = _wrap_idx(
                qidx[c][s * 128:(s + kb) * 128])

    nf = np.zeros((NPAD, 3), np.float32)
    nf[:N] = node_features
    nf1T = np.zeros((NCORES, 4, NSH), np.float32)
    for c in range(NCORES):
        nf1T[c, 0:3] = nf[c * NSH:(c + 1) * NSH].T
        nf1T[c, 3] = 1.0

    sched = dict(K=K, C=C, batches=batches)
    percore = dict(qidx_w=qidx_w,
                   rloc=rloc.astype(BF16),
                   rep8=rep8,
                   ef=ef.astype(BF16),
                   nf1T=nf1T.astype(BF16))
    return sched, percore


def _build(sched):
    """Build the Bass program for the shared chunk schedule."""
    import concourse.mybir as mybir
    import concourse.tile as tile
    from concourse import bacc

    dt = mybir.dt
    fp = dt.float32
    bf = dt.bfloat16
    AOT = mybir.AluOpType
    ACT = mybir.ActivationFunctionType

    C = sched["C"]
    batches = sched["batches"]
    K = sched["K"]

    nc = bacc.Bacc("TRN2", num_devices=NCORES, num_swdge_queues=4)

    # ---- I/O ----
    nf1T_d = nc.dram_tensor("nf1T", [4, NSH], bf, kind="ExternalInput")
    qidx_d = nc.dram_tensor("qidx", [128, C * 8], dt.int16, kind="ExternalInput")
    rloc_d = nc.dram_tensor("rloc", [128, C], bf, kind="ExternalInput")
    rep8_d = nc.dram_tensor("rep8", [GW, C * 128], dt.int8, kind="ExternalInput")
    ef_d = nc.dram_tensor("ef", [2, C * 128], bf, kind="ExternalInput")
    iotap8_d = nc.dram_tensor("iotap8", [128, 1], dt.int8, kind="ExternalInput")
    iota_d = nc.dram_tensor("iota", [128, GW], bf, kind="ExternalInput")
    iotap_d = nc.dram_tensor("iotap", [128, 1], bf, kind="ExternalInput")
    onesbd_d = nc.dram_tensor("onesbd", [128, 2], bf, kind="ExternalInput")
    ones64_d = nc.dram_tensor("ones64", [1, 64], bf, kind="ExternalInput")
    encw1b_d = nc.dram_tensor("encw1b", [4, HID], bf, kind="ExternalInput")
    encw2_d = nc.dram_tensor("encw2", [HID, F], bf, kind="ExternalInput")
    encb2_d = nc.dram_tensor("encb2", [F, 1], fp, kind="ExternalInput")
    wrb1_d = nc.dram_tensor("wrb1", [L, 65, F], bf, kind="ExternalInput")
    wc_d = nc.dram_tensor("wc", [L, 65, F], bf, kind="ExternalInput")
    web_d = nc.dram_tensor("web", [L, 2, F], bf, kind="ExternalInput")
    webrep_d = nc.dram_tensor("webrep", [L, 2, G * F], bf, kind="ExternalInput")
    w2b_d = nc.dram_tensor("w2b", [L, 65, F], bf, kind="ExternalInput")
    skb_d = nc.dram_tensor("skb", [L, F, 1], fp, kind="ExternalInput")
    skw_d = nc.dram_tensor("skw", [L, F, F], bf, kind="ExternalInput")
    skwb_d = nc.dram_tensor("skwb", [L, 65, F], bf, kind="ExternalInput")
    ident_d = nc.dram_tensor("ident", [128, 128], bf, kind="ExternalInput")
    lng_d = nc.dram_tensor("lng", [L, F, 1], fp, kind="ExternalInput")
    lnb_d = nc.dram_tensor("lnb", [L, F, 1], fp, kind="ExternalInput")
    hw1_d = nc.dram_tensor("hw1", [F, F], bf, kind="ExternalInput")
    hb1_d = nc.dram_tensor("hb1", [F, 1], fp, kind="ExternalInput")
    hw2_d = nc.dram_tensor("hw2", [F, POLY], bf, kind="ExternalInput")
    hb2_d = nc.dram_tensor("hb2", [POLY, 1], fp, kind="ExternalInput")
    outT_d = nc.dram_tensor("outT", [POLY, NSH], fp, kind="ExternalOutput")
    # internal (bf16 pair layout: row j holds nodes 2j, 2j+1)
    q_local = nc.dram_tensor("q_local", [NSH // 2, 2 * F], bf)
    q_full = nc.dram_tensor("q_full", [NPAIR, 2 * F], bf, addr_space="Shared")

    ntiles = [(t * TN, min(TN, NSH - t * TN)) for t in range((NSH + TN - 1) // TN)]
    n128 = [(t * 128, min(128, NSH - t * 128)) for t in range((NSH + 127) // 128)]

    with tile.TileContext(nc) as tc:
        with (
            tc.tile_pool(name="persist", bufs=1) as pp,
            tc.tile_pool(name="wts", bufs=1) as wp,
        ):
            # persistent state
            hT = pp.tile([65, NSH], bf)         # rows 0-63 h, row 64 ones
            aggT = pp.tile([65, NSH], bf)       # rows 0-63 agg, row 64 deg
            PW_all = pp.tile([128, G * F], bf)  # rows 0:126 P_g, 126:128 W_e
            iota_t = pp.tile([128, GW], bf)
            iotap_t = pp.tile([128, 1], bf)
            iotap8_t = pp.tile([128, 1], dt.int8)
            nc.sync.dma_start(out=iotap8_t[:], in_=iotap8_d[:, :])
            ident_t = pp.tile([128, 128], bf)
            nc.sync.dma_start(out=ident_t[:], in_=ident_d[:, :])
            onesbd_t = pp.tile([128, 2], bf)
            ones64_t = pp.tile([1, 64], bf)
            nc.sync.dma_start(out=iota_t[:], in_=iota_d[:, :])
            nc.sync.dma_start(out=iotap_t[:], in_=iotap_d[:, :])
            nc.sync.dma_start(out=onesbd_t[:], in_=onesbd_d[:, :])
            nc.sync.dma_start(out=ones64_t[:], in_=ones64_d[:, :])
            nc.vector.memset(hT[64:65, :], 1.0)
            eps_t = pp.tile([1, 1], fp)
            nc.vector.memset(eps_t[:], 1e-5)

            # weights resident
            encw1b_t = wp.tile([4, HID], bf)
            encw2_t = wp.tile([HID, F], bf)
            encb2_t = wp.tile([F, 1], fp)
            nc.sync.dma_start(out=encw1b_t[:], in_=encw1b_d[:, :])
            nc.sync.dma_start(out=encw2_t[:], in_=encw2_d[:, :])
            nc.sync.dma_start(out=encb2_t[:], in_=encb2_d[:, :])
            wrb1_t = [wp.tile([65, F], bf, name=f"wrb1{l}") for l in range(L)]
            wc_t = [wp.tile([65, F], bf, name=f"wc{l}") for l in range(L)]
            web_t = [wp.tile([2, F], bf, name=f"web{l}") for l in range(L)]
            w2b_t = [wp.tile([65, F], bf, name=f"w2b{l}") for l in range(L)]
            skb_t = [wp.tile([F, 1], fp, name=f"skb{l}") for l in range(L)]
            skw_t = [wp.tile([F, F], bf, name=f"skw{l}") for l in range(L)]
            skwb_t = [wp.tile([65, F], bf, name=f"skwb{l}") for l in range(L)]
            lng_t = [wp.tile([F, 1], fp, name=f"lng{l}") for l in range(L)]
            lnb_t = [wp.tile([F, 1], fp, name=f"lnb{l}") for l in range(L)]
            for l in range(L):
                nc.sync.dma_start(out=wrb1_t[l][:], in_=wrb1_d[l, :, :])
                nc.sync.dma_start(out=wc_t[l][:], in_=wc_d[l, :, :])
                nc.sync.dma_start(out=web_t[l][:], in_=web_d[l, :, :])
                nc.sync.dma_start(out=w2b_t[l][:], in_=w2b_d[l, :, :])
                nc.sync.dma_start(out=skb_t[l][:], in_=skb_d[l, :, :])
                nc.sync.dma_start(out=skw_t[l][:], in_=skw_d[l, :, :])
                nc.sync.dma_start(out=skwb_t[l][:], in_=skwb_d[l, :, :])
                nc.sync.dma_start(out=lng_t[l][:], in_=lng_d[l, :, :])
                nc.sync.dma_start(out=lnb_t[l][:], in_=lnb_d[l, :, :])
            hw1_t = wp.tile([F, F], bf)
            hb1_t = wp.tile([F, 1], fp)
            hw2_t = wp.tile([F, POLY], bf)
            hb2_t = wp.tile([POLY, 1], fp)
            nc.sync.dma_start(out=hw1_t[:], in_=hw1_d[:, :])
            nc.sync.dma_start(out=hb1_t[:], in_=hb1_d[:, :])
            nc.sync.dma_start(out=hw2_t[:], in_=hw2_d[:, :])
            nc.sync.dma_start(out=hb2_t[:], in_=hb2_d[:, :])

            # ---------------- encoder ----------------
            with (
                tc.tile_pool(name="enc_sb", bufs=2) as esb,
                tc.tile_pool(name="enc_nf", bufs=1) as enf,
                tc.tile_pool(name="enc_ps", bufs=2, space="PSUM") as eps,
            ):
                nf_t = enf.tile([4, NSH], bf)
                nc.sync.dma_start(out=nf_t[:], in_=nf1T_d[:, :])
                for (t0, tw) in ntiles:
                    hid_ps = eps.tile([HID, TN], fp, tag="hid")
                    nc.tensor.matmul(out=hid_ps[:, :tw], lhsT=encw1b_t[:],
                                     rhs=nf_t[:, t0:t0 + tw], start=True, stop=True)
                    hid_sb = esb.tile([HID, TN], bf, tag="hsb")
                    nc.vector.tensor_scalar(out=hid_sb[:, :tw], in0=hid_ps[:, :tw],
                                            scalar1=0.0, scalar2=None, op0=AOT.max)
                    h_ps = eps.tile([F, TN], fp, tag="hps")
                    nc.tensor.matmul(out=h_ps[:, :tw], lhsT=encw2_t[:],
                                     rhs=hid_sb[:, :tw], start=True, stop=True)
                    nc.vector.tensor_scalar(out=hT[0:F, t0:t0 + tw], in0=h_ps[:, :tw],
                                            scalar1=encb2_t[:, 0:1], scalar2=None,
                                            op0=AOT.add)

            # ---------------- layers ----------------
            for l in range(L):
                # Q compute per group -> q_local -> AllGather; P overlaps
                with (
                    tc.tile_pool(name=f"pq_sb{l}", bufs=3) as qsb,
                    tc.tile_pool(name=f"pq_ps{l}", bufs=3, space="PSUM") as qps,
                ):
                    for g8 in range(0, G, 8):
                        w = min(8, G - g8)
                        q8_ps = qps.tile([GW, 512], fp, tag="qps")
                        for j in range(w):
                            g = g8 + j
                            nc.tensor.matmul(out=q8_ps[:, j * F:(j + 1) * F],
                                             lhsT=hT[:, g * GW:(g + 1) * GW],
                                             rhs=wc_t[l][:], start=True, stop=True)
                        q8_sb = qsb.tile([GW, 512], bf, tag="qsb")
                        nc.vector.tensor_copy(out=q8_sb[:, 0:w * F],
                                              in_=q8_ps[:, 0:w * F])
                        for j in range(w):
                            g = g8 + j
                            nc.sync.dma_start(
                                out=q_local[g * (GW // 2):(g + 1) * (GW // 2), :],
                                in_=q8_sb[:, j * F:(j + 1) * F])

                    nc.gpsimd.collective_compute(
                        "AllGather", AOT.bypass,
                        replica_groups=[list(range(NCORES))],
                        ins=[q_local[:, :]], outs=[q_full[:, :]],
                    )

                    # P tables (overlap the collective)
                    nc.sync.dma_start(out=PW_all[GW:128, :],
                                      in_=webrep_d[l, :, :])
                    for g8 in range(0, G, 8):
                        w = min(8, G - g8)
                        p8_ps = qps.tile([GW, 512], fp, tag="pps")
                        for j in range(w):
                            g = g8 + j
                            nc.tensor.matmul(out=p8_ps[:, j * F:(j + 1) * F],
                                             lhsT=hT[:, g * GW:(g + 1) * GW],
                                             rhs=wrb1_t[l][:], start=True, stop=True)
                        nc.vector.tensor_copy(
                            out=PW_all[0:GW, g8 * F:(g8 + w) * F],
                            in_=p8_ps[:, 0:w * F])

                # edge phase
                with (
                    tc.tile_pool(name=f"nu_sb{l}", bufs=2) as nsb,
                    tc.tile_pool(name=f"nu_ps{l}", bufs=2, space="PSUM") as nps,
                    tc.tile_pool(name=f"nu_psT{l}", bufs=2, space="PSUM") as npsT,
                    tc.tile_pool(name=f"eg_ix{l}", bufs=8) as gsb,
                    tc.tile_pool(name=f"eg_qg{l}", bufs=8) as qgp,
                    tc.tile_pool(name=f"eg_oh{l}", bufs=3) as ohp,
                    tc.tile_pool(name=f"eg_ohn{l}", bufs=3) as ohnp,
                    tc.tile_pool(name=f"eg_msg{l}", bufs=3) as msb,
                    tc.tile_pool(name=f"eg_ps{l}", bufs=2, space="PSUM") as zps,
                    tc.tile_pool(name=f"agg_ps{l}", bufs=2, space="PSUM") as aps,
                ):
                    def node_tile(t0, tw):
                        sl = slice(t0, t0 + tw)
                        x_ps = nps.tile([128, F], fp, tag="hn")
                        nc.tensor.matmul(out=x_ps[:tw, :], lhsT=aggT[:, sl],
                                         rhs=w2b_t[l][:], start=True, stop=False)
                        nc.tensor.matmul(out=x_ps[:tw, :], lhsT=hT[:, sl],
                                         rhs=skwb_t[l][:], start=False, stop=True)
                        xsq = nsb.tile([128, F], bf, tag="xsq")
                        nc.scalar.activation(out=xsq[:tw, :], in_=x_ps[:tw, :],
                                             func=ACT.Square)
                        s1 = nsb.tile([128, 1], fp, tag="s1")
                        s2 = nsb.tile([128, 1], fp, tag="s2")
                        nc.vector.tensor_reduce(out=s1[:tw], in_=x_ps[:tw, :],
                                                axis=mybir.AxisListType.X,
                                                op=AOT.add)
                        nc.vector.tensor_reduce(out=s2[:tw], in_=xsq[:tw, :],
                                                axis=mybir.AxisListType.X,
                                                op=AOT.add)
                        negmu = nsb.tile([128, 1], fp, tag="negmu")
                        var = nsb.tile([128, 1], fp, tag="var")
                        tmp = nsb.tile([128, 1], fp, tag="tmp")
                        nc.vector.tensor_scalar(out=negmu[:tw], in0=s1[:tw],
                                                scalar1=-1.0 / F, scalar2=None,
                                                op0=AOT.mult)
                        nc.vector.scalar_tensor_tensor(
                            out=tmp[:tw], in0=negmu[:tw], scalar=-1.0,
                            in1=negmu[:tw], op0=AOT.mult, op1=AOT.mult)
                        nc.vector.tensor_scalar(out=var[:tw], in0=s2[:tw],
                                                scalar1=1.0 / F, scalar2=1e-5,
                                                op0=AOT.mult, op1=AOT.add)
                        nc.vector.tensor_tensor(out=var[:tw], in0=var[:tw],
                                                in1=tmp[:tw], op=AOT.add)
                        nc.scalar.activation(out=var[:tw], in_=var[:tw],
                                             func=ACT.Sqrt)
                        nc.vector.reciprocal(out=var[:tw], in_=var[:tw])
                        xn = nsb.tile([128, F], bf, tag="xn")
                        nc.vector.tensor_scalar(out=xn[:tw, :], in0=x_ps[:tw, :],
                                                scalar1=negmu[:tw, 0:1],
                                                scalar2=var[:tw, 0:1],
                                                op0=AOT.add, op1=AOT.mult)
                        yT_ps = npsT.tile([F, 128], bf, tag="yT")
                        nc.tensor.transpose(out=yT_ps[:, :tw], in_=xn[:tw, :],
                                            identity=ident_t[0:tw, 0:tw])
                        nc.scalar.activation(out=hT[0:F, sl], in_=yT_ps[:, :tw],
                                             func=ACT.Relu,
                                             bias=lnb_t[l][:, 0:1],
                                             scale=lng_t[l][:, 0:1])

                    next_tile = 0
                    for bi, b in enumerate(batches):
                        kb, keb, s = b["kb"], b["keb"], b["c0"]
                        cgrp = b["cgrp"]
                        qidx_t = gsb.tile([128, kb * 8], dt.int16, tag="qidx")
                        rloc_t = gsb.tile([128, kb], bf, tag="rloc")
                        nc.sync.dma_start(out=qidx_t[:, :],
                                          in_=qidx_d[:, s * 8:(s + kb) * 8])
                        nc.sync.dma_start(out=rloc_t[:, :], in_=rloc_d[:, s:s + kb])

                        # bf16 pair gather: one call per batch, rotating queues
                        qg = qgp.tile([128, kb, 2 * F], bf, tag="qg")
                        nc.gpsimd.dma_gather(
                            out_ap=qg[:], in_ap=q_full[:, :], idxs_ap=qidx_t[:],
                            num_idxs=kb * 128, num_idxs_reg=kb * 128,
                            elem_size=2 * F, single_packet=False,
                            queue_num=bi % 4)

                        # combined [one-hot; ef] tile: one-hot from int8 rep
                        ohnef_t = ohnp.tile([128, kb * 128], bf, tag="ohnef")
                        rep_t = gsb.tile([GW, kb * 128], dt.int8, tag="rep8")
                        nc.sync.dma_start(out=rep_t[:, :],
                                          in_=rep8_d[:, s * 128:(s + kb) * 128])
                        nc.sync.dma_start(out=ohnef_t[GW:128, :],
                                          in_=ef_d[:, s * 128:(s + kb) * 128])
                        nc.vector.tensor_tensor(
                            out=ohnef_t[0:GW, :],
                            in0=rep_t[:, :],
                            in1=iotap8_t[0:GW, 0:1].to_broadcast([GW, kb * 128]),
                            op=AOT.is_equal)

                        # one-hot (edge-partition) for scatter
                        oh_t = ohp.tile([128, kb, GW], bf, tag="oh")
                        nc.vector.tensor_tensor(
                            out=oh_t[:],
                            in0=rloc_t[:, :, None].to_broadcast([128, kb, GW]),
                            in1=iota_t[:, None, :].to_broadcast([128, kb, GW]),
                            op=AOT.is_equal)

                        # pg + ef in one matmul per chunk; msg per slab of 8
                        msg_t = msb.tile([128, kb, F + 1], bf, tag="msg")
                        nc.vector.memset(msg_t[:, :, F:F + 1], 1.0)
                        for (c0, c1, qoff) in ((0, keb, 0), (keb, kb, F)):
                            for s8 in range(c0, c1, 8):
                                w8 = min(8, c1 - s8)
                                pgz_ps = zps.tile([128, 512], fp, tag="z")
                                for j in range(w8):
                                    c = s8 + j
                                    g = int(cgrp[c])
                                    nc.tensor.matmul(
                                        out=pgz_ps[:, j * F:(j + 1) * F],
                                        lhsT=ohnef_t[:, c * 128:(c + 1) * 128],
                                        rhs=PW_all[:, g * F:(g + 1) * F],
                                        start=True, stop=True)
                                nc.vector.tensor_tensor(
                                    out=msg_t[:, s8:s8 + w8, 0:F],
                                    in0=pgz_ps[:, 0:w8 * F].rearrange(
                                        "p (c f) -> p c f", f=F),
                                    in1=qg[:, s8:s8 + w8, qoff:qoff + F],
                                    op=AOT.add)
                                nc.scalar.activation(
                                    out=msg_t[:, s8:s8 + w8, 0:F],
                                    in_=msg_t[:, s8:s8 + w8, 0:F], func=ACT.Relu)

                        # scatter per group
                        for g in b["groups"]:
                            kg = int(K[g])
                            if kg == 0:
                                continue
                            ranges = [b["epos"][g], b["opos"][g]]
                            ranges = [(a, z) for (a, z) in ranges if z > a]
                            agg_ps = aps.tile([F + 1, GW], fp, tag="agg")
                            ci = 0
                            for (a, z) in ranges:
                                for c in range(a, z):
                                    nc.tensor.matmul(
                                        out=agg_ps[:],
                                        lhsT=msg_t[:, c, :],
                                        rhs=oh_t[:, c, :],
                                        start=(ci == 0), stop=(ci == kg - 1))
                                    ci += 1
                            nc.vector.tensor_copy(
                                out=aggT[0:F + 1, g * GW:(g + 1) * GW],
                                in_=agg_ps[:])

                        gdone = min((bi + 1) * GB, G) - 1
                        while next_tile < len(n128):
                            t0, tw = n128[next_tile]
                            gneed = (t0 + tw + GW - 1) // GW - 1
                            if gneed > gdone:
                                break
                            node_tile(t0, tw)
                            next_tile += 1
                    assert next_tile == len(n128)

            # ---------------- head ----------------
            with (
                tc.tile_pool(name="hd_sb", bufs=2) as hsb,
                tc.tile_pool(name="hd_ps", bufs=2, space="PSUM") as hps,
            ):
                for (t0, tw) in ntiles:
                    sl = slice(t0, t0 + tw)
                    z_ps = hps.tile([F, TN], fp, tag="z1")
                    nc.tensor.matmul(out=z_ps[:, :tw], lhsT=hw1_t[:],
                                     rhs=hT[0:F, sl], start=True, stop=True)
                    z_sb = hsb.tile([F, TN], bf, tag="z1sb")
                    nc.vector.tensor_scalar(out=z_sb[:, :tw], in0=z_ps[:, :tw],
                                            scalar1=hb1_t[:, 0:1], scalar2=0.0,
                                            op0=AOT.add, op1=AOT.max)
                    o_ps = hps.tile([POLY, TN], fp, tag="ops")
                    nc.tensor.matmul(out=o_ps[:, :tw], lhsT=hw2_t[:],
                                     rhs=z_sb[:, :tw], start=True, stop=True)
                    o_sb = hsb.tile([POLY, TN], fp, tag="osb")
                    nc.vector.tensor_scalar(out=o_sb[:, :tw], in0=o_ps[:, :tw],
                                            scalar1=hb2_t[:, 0:1], scalar2=None,
                                            op0=AOT.add)
                    nc.sync.dma_start(out=outT_d[:, t0:t0 + tw], in_=o_sb[:, :tw])

    nc.compile()
    return nc


def _run(inputs, trace=False):
    from concourse import bass_utils

    node_features = np.asarray(inputs["node_features"], np.float32)
    edge_index = np.asarray(inputs["edge_index"])
    edge_features = np.asarray(inputs["edge_features"], np.float32)

    sched, percore = _preprocess(node_features, edge_index, edge_features)
    nc = _build(sched)

    # ---- weights (host prep) ----
    s = np.float32
    enc_w1 = np.asarray(inputs["enc_w1"], s)
    enc_b1 = np.asarray(inputs["enc_b1"], s)
    enc_w2 = np.asarray(inputs["enc_w2"], s)
    enc_b2 = np.asarray(inputs["enc_b2"], s)
    conv_w1 = np.asarray(inputs["conv_w1"], s)
    conv_b1 = np.asarray(inputs["conv_b1"], s)
    conv_w2 = np.asarray(inputs["conv_w2"], s)
    conv_b2 = np.asarray(inputs["conv_b2"], s)
    skip_w = np.asarray(inputs["skip_w"], s)
    skip_b = np.asarray(inputs["skip_b"], s)
    ln_g = np.asarray(inputs["ln_g"], s)
    ln_b = np.asarray(inputs["ln_b"], s)
    head_w1 = np.asarray(inputs["head_w1"], s)
    head_b1 = np.asarray(inputs["head_b1"], s)
    head_w2 = np.asarray(inputs["head_w2"], s)
    head_b2 = np.asarray(inputs["head_b2"], s)

    encw1b = np.concatenate([enc_w1, enc_b1[None, :]], axis=0)
    wrb1 = np.concatenate([conv_w1[:, 0:F, :], conv_b1[:, None, :]], axis=1)
    wc = np.concatenate([conv_w1[:, F:2 * F, :],
                         np.zeros((L, 1, F), s)], axis=1)
    web = conv_w1[:, 2 * F:2 * F + 2, :]
    w2b = np.concatenate([conv_w2, conv_b2[:, None, :]], axis=1)

    skwb = np.concatenate([skip_w, skip_b[:, None, :]], axis=1)
    ident = np.eye(128, dtype=s)
    iota = np.tile(np.arange(GW, dtype=s), (128, 1))
    iotap = np.arange(128, dtype=s).reshape(128, 1)
    onesbd = np.zeros((128, 2), s)
    onesbd[0:F, 0] = 1.0 / F
    onesbd[F:2 * F, 1] = 1.0 / F
    ones64 = np.ones((1, F), s)

    b = lambda a: np.ascontiguousarray(a.astype(BF16))
    shared = dict(
        iota=b(iota), iotap=b(iotap),
        iotap8=iotap.astype(np.int8), onesbd=b(onesbd), ones64=b(ones64),
        encw1b=b(encw1b), encw2=b(enc_w2), encb2=enc_b2.reshape(F, 1),
        wrb1=b(wrb1), wc=b(wc), skwb=b(skwb), ident=b(ident), web=b(web),
        webrep=b(np.tile(web, (1, 1, G)).reshape(L, 2, G * F)),
        w2b=b(w2b), skw=b(skip_w),
        skb=skip_b.reshape(L, F, 1),
        lng=ln_g.reshape(L, F, 1), lnb=ln_b.reshape(L, F, 1),
        hw1=b(head_w1), hb1=head_b1.reshape(F, 1),
        hw2=b(head_w2), hb2=head_b2.reshape(POLY, 1),
    )
    in_maps = []
    for c in range(NCORES):
        m = dict(shared)
        m["nf1T"] = percore["nf1T"][c]
        m["qidx"] = percore["qidx_w"][c]
        m["rloc"] = percore["rloc"][c]
        m["rep8"] = percore["rep8"][c]
        m["ef"] = percore["ef"][c]
        in_maps.append(m)

    res = bass_utils.run_bass_kernel_spmd(
        nc, in_maps, core_ids=list(range(NCORES)), trace=trace)
    outs = res.results
    full = np.concatenate([outs[c]["outT"].T for c in range(NCORES)], axis=0)
    return full[:N].astype(np.float32), res


def kernel(**inputs) -> np.ndarray:
    out, _ = _run(inputs, trace=False)
    return out
